# revision 15
# baseline (speedup 1.0000x reference)
"""nn_Intra_ResNet Trainium2 kernel — full inputs -> full output, 8-core SPMD.

Row-shards the 384x384 grid (48 rows/core). Convs are bf16 matmuls on the
TensorEngine (channels on partitions, tap-paired to K=128 via a row-shifted
replica in the upper 64 partitions). InstanceNorm stats: per-row bn_stats ->
bn_aggr -> tiny AllReduce; normalization+leaky-relu fused into one ScalarE
Lrelu op with per-channel scale/bias. Halo rows (width 4) exchanged every
stage via AllGather of raw boundary strips + per-partition indirect DMA.
"""
import numpy as np
import ml_dtypes

import concourse.bass as bass
import concourse.tile as tile
import concourse.mybir as mybir
from concourse.bass import IndirectOffsetOnAxis
from concourse.bass_utils import run_bass_kernel_spmd

F32 = mybir.dt.float32
F32R = mybir.dt.float32r
BF16 = mybir.dt.bfloat16
I32 = mybir.dt.int32
AF = mybir.ActivationFunctionType
ALU = mybir.AluOpType

NCORES = 8
C = 64
L = 384
R = 48            # rows per core
H = 4             # halo width (max dilation)
RT = R + 2 * H    # 56 rows in tiles
WC = L + 2 * H    # 392 cols (padded)
EPS = 1e-5
D1, D2 = 788, 210
D1P, D2P = 896, 256   # padded (7*128, 2*128)
DIL = (1, 2, 4, 2, 1)
CONV_D = [DIL[s // 2] for s in range(10)]
NEXT_D = [CONV_D[s + 1] for s in range(9)] + [None]
ALPHA = 0.01

# halo width applied to each stage's output: H(s) = max(next conv's
# dilation, halo needed when this tensor is the residual later).
HW_OUT = [0] * 10
HW_OUT[9] = 0
for _s in (7, 5, 3, 1):
    HW_OUT[_s] = max(CONV_D[_s + 1], HW_OUT[_s + 2] if _s + 2 <= 9 else 0)
for _s in (0, 2, 4, 6, 8):
    HW_OUT[_s] = CONV_D[_s + 1]
HW_X0 = max(CONV_D[0], HW_OUT[1])
# exchange widths, in emission order: x0's, then stage 0..8 outputs
HX_W = [HW_X0] + [HW_OUT[_s] for _s in range(9)]
import os as _os
if _os.environ.get("HALO_UNIFORM", "0") == "1":
    HW_OUT = [4] * 9 + [0]
    HW_X0 = 4
    HX_W = [4] * 10

# apply chunks (tile rows, max 8 rows each); 'T'/'B' are masked halo chunks
def chunks_for(w):
    ch = [(4, 12, 'A')]
    if w:
        ch.insert(0, (4 - w, 4, 'T'))
        ch.append((52, 52 + w, 'B'))
    ch += [(44, 52, 'A'), (12, 20, 'A'), (20, 28, 'A'), (28, 36, 'A'),
           (36, 44, 'A')]
    return ch
FINAL_CHUNKS = [(4 + 4 * i, 8 + 4 * i) for i in range(12)]

_KEEP = ("InstEventSemaphore", "InstNoOp")


def split_multi_waits(nc):
    """This walrus build accepts at most 1 sem wait per instruction:
    hoist extra waits onto preceding same-engine NoOps."""
    n = 0
    for f in nc.m.functions:
        for bb in f.blocks:
            out = []
            for inst in bb.instructions:
                si = inst.sync_info
                if (si is not None and si.on_wait and len(si.on_wait) > 1
                        and type(inst).__name__ not in _KEEP):
                    waits = list(si.on_wait)
                    for w in waits[:-1]:
                        n += 1
                        nop = mybir.InstNoOp(name=f"waitnop_{n}")
                        nop.engine = inst.engine
                        nop.sync_info = mybir.SyncInfo(on_wait=[w], on_update=[])
                        out.append(nop)
                    inst.sync_info = mybir.SyncInfo(
                        on_wait=[waits[-1]], on_update=list(si.on_update))
                out.append(inst)
            bb.instructions = out
    return n


def build():
    nc = bass.Bass()
    P = lambda n, sh, dt: nc.declare_dram_parameter(n, sh, dt, isOutput=False)
    x1p = P("x1p", [D1P, L], F32)
    x1loc = P("x1loc", [D1P, R], F32)
    x2a = P("x2a", [128, R * L], F32)
    x2b = P("x2b", [128, R * L], F32)
    w1aT = P("w1aT", [D1P, C], F32)
    w1bT = P("w1bT", [D1P, C], F32)
    w2T = P("w2T", [D2P, C], F32)
    w3t_p = P("w3t", [C, 2 * C], BF16)
    wpair_p = P("wpair", [128, 10 * 3 * C], BF16)
    wsing_p = P("wsing", [C, 10 * 3 * C], BF16)
    gb_p = P("gb", [C, 26], F32)
    masks_p = P("masks", [C, 2], F32)
    hidx_p = P("hidx", [C, 2], I32)
    out_p = nc.declare_dram_parameter("out", [C, R * L], F32, isOutput=True)

    groups = [list(range(NCORES))]

    with tile.TileContext(nc) as tc:
        with (
            tc.tile_pool(name="wp", bufs=1) as wp,
            tc.tile_pool(name="act", bufs=3) as actp,
            tc.tile_pool(name="rawp", bufs=1) as rawp,
            tc.tile_pool(name="misc", bufs=2) as miscp,
            tc.tile_pool(name="x1w", bufs=2) as x1wp,
            tc.tile_pool(name="stg", bufs=2) as stgp,
            tc.tile_pool(name="svp", bufs=12) as svp,
            tc.tile_pool(name="psum", bufs=8, space="PSUM") as pp,
            tc.tile_pool(name="dram", bufs=1, space="DRAM") as dp,
        ):
            # ---------- resident small tensors ----------
            wpair_t = wp.tile([128, 10 * 3 * C], BF16, tag="wpair", name="wpair_t")
            wsing_t = wp.tile([C, 10 * 3 * C], BF16, tag="wsing", name="wsing_t")
            w3t = wp.tile([C, 2 * C], BF16, tag="w3t", name="w3t_t")
            gbt = wp.tile([C, 26], F32, tag="gbt", name="gbt_t")
            maskt = wp.tile([C, 2], F32, tag="maskt", name="maskt_t")
            idxt = wp.tile([C, 2], I32, tag="idxt", name="idxt_t")
            nc.sync.dma_start(wpair_t[:], wpair_p[:])
            nc.sync.dma_start(wsing_t[:], wsing_p[:])
            nc.sync.dma_start(w3t[:], w3t_p[:])
            nc.sync.dma_start(gbt[:], gb_p[:])
            nc.sync.dma_start(maskt[:], masks_p[:])
            nc.sync.dma_start(idxt[:], hidx_p[:])

            raw = rawp.tile([C, RT * L], BF16, tag="raw", name="raw_t")
            rawv = raw[:].rearrange("p (r c) -> p r c", c=L)

            # dram bounce buffers
            cc64i = dp.tile([C, 2], F32, tag="cc64i", name="cc64i")
            cc64o = dp.tile([C, 2], F32, tag="cc64o", name="cc64o")
            cc128i = dp.tile([128, 4], F32, tag="cc128i", name="cc128i")
            cc128o = dp.tile([128, 4], F32, tag="cc128o", name="cc128o")
            stripb_w = {w: dp.tile([C, 2, w * L], BF16, tag=f"stripb{w}",
                                   name=f"stripb{w}") for w in (1, 2, 4)}
            # one AllGather output per exchange (Tile: single writer per
            # Shared DRAM tensor). Edge cores read slot 0 garbage; the
            # masked apply zeroes those rows.
            recv_tiles = [
                dp.tile([8, C, 2, HX_W[k] * L], BF16, tag=f"recv{k}",
                        name=f"recv{k}", addr_space="Shared")
                for k in range(10)]

            # =========================================================
            # helpers
            # =========================================================
            sv_count = [0]

            def small_vecs(n=16, p=C):
                sv_count[0] += 1
                return svp.tile([p, n], F32, tag="sv", name=f"sv{sv_count[0]}")

            def rsqrt_cols(dst, src):
                """dst = 1/sqrt(src + EPS); [P,1] f32 slices."""
                nc.vector.tensor_scalar(dst, src, float(EPS), None, ALU.add)
                nc.scalar.activation(dst, dst, AF.Sqrt)
                nc.vector.reciprocal(dst, dst)

            ar_count = [0]

            def emit_stats_ar(stg_t, nrows, gamma, beta, masked):
                """stats groups [C, nrows, 6] -> AllReduce -> per-channel
                scale/bias (+ masked variants). vec cols: 0=s 1=b 2=sT 3=bT
                4=sB 5=bB."""
                ar_count[0] += 1
                v = small_vecs()
                mv = small_vecs(4)
                nc.vector.bn_aggr(mv[:, 0:2], stg_t[:].rearrange(
                    "p (r s) -> p r s", s=6)[:, 0:nrows, :])
                pl = small_vecs(2)
                nc.vector.tensor_scalar(pl[:, 0:1], mv[:, 0:1], 0.125, None, ALU.mult)
                nc.vector.scalar_tensor_tensor(
                    mv[:, 2:3], mv[:, 0:1], 1.0, mv[:, 0:1], ALU.mult, ALU.mult)
                nc.vector.scalar_tensor_tensor(
                    mv[:, 3:4], mv[:, 2:3], 1.0, mv[:, 1:2], ALU.mult, ALU.add)
                nc.vector.tensor_scalar(pl[:, 1:2], mv[:, 3:4], 0.125, None, ALU.mult)
                nc.sync.dma_start(cc64i[:], pl[:])
                nc.gpsimd.collective_compute(
                    "AllReduce", ALU.add, replica_groups=groups,
                    ins=[cc64i.opt()], outs=[cc64o.opt()])
                g = small_vecs(4)  # 0=mu_g 1=E2_g 2=var_g 3=tmp
                nc.sync.dma_start(g[:, 0:2], cc64o[:])
                nc.vector.scalar_tensor_tensor(
                    g[:, 3:4], g[:, 0:1], 1.0, g[:, 0:1], ALU.mult, ALU.mult)
                nc.vector.scalar_tensor_tensor(
                    g[:, 2:3], g[:, 3:4], -1.0, g[:, 1:2], ALU.mult, ALU.add)
                rsqrt_cols(v[:, 6:7], g[:, 2:3])
                nc.vector.scalar_tensor_tensor(
                    v[:, 0:1], v[:, 6:7], 1.0, gamma, ALU.mult, ALU.mult)
                nc.vector.scalar_tensor_tensor(
                    v[:, 7:8], g[:, 0:1], 1.0, v[:, 0:1], ALU.mult, ALU.mult)
                nc.vector.scalar_tensor_tensor(
                    v[:, 1:2], v[:, 7:8], -1.0, beta, ALU.mult, ALU.add)
                if masked:
                    for (col, src, mc) in ((2, 0, 0), (3, 1, 0), (4, 0, 1), (5, 1, 1)):
                        nc.vector.scalar_tensor_tensor(
                            v[:, col:col + 1], v[:, src:src + 1], 1.0,
                            maskt[:, mc:mc + 1], ALU.mult, ALU.mult)
                return v

            hx_count = [0]

            def emit_halo_collective():
                w = HX_W[hx_count[0]]
                sb = stripb_w[w]
                nc.sync.dma_start(
                    sb[:, 0], rawv[:, 4:4 + w, :].rearrange("p r c -> p (r c)"))
                nc.sync.dma_start(
                    sb[:, 1], rawv[:, 52 - w:52, :].rearrange("p r c -> p (r c)"))
                nc.gpsimd.collective_compute(
                    "AllGather", ALU.bypass, replica_groups=groups,
                    ins=[sb.opt()], outs=[recv_tiles[hx_count[0]].opt()])

            def emit_halo_gather():
                w = HX_W[hx_count[0]]
                rcv = recv_tiles[hx_count[0]]
                hx_count[0] += 1
                for k, rows in ((0, raw[:, (4 - w) * L:4 * L]),
                                (1, raw[:, 52 * L:(52 + w) * L])):
                    nc.gpsimd.indirect_dma_start(
                        out=rows,
                        out_offset=None,
                        in_=rcv[:].rearrange("s p t f -> (s p t) f"),
                        in_offset=IndirectOffsetOnAxis(ap=idxt[:, k:k + 1], axis=0),
                    )

            def emit_apply(v, X_out, d_next, res=None, halo_w=H):
                """raw [C, RT, L] -> X_out rows, cols 4:388 (bf16), with
                per-chunk replica DMA into the upper 64 partitions. The two
                interior chunks that gate the next conv's first rows go to
                DVE (3-op lrelu) on non-residual stages so ACT and DVE work
                the post-AllReduce apply in parallel."""
                xo = X_out[:].rearrange("p (r c) -> p r c", c=WC)
                dve_chunks = {(4, 12), (44, 52)} if res is None else set()
                for (rs, re, kind) in chunks_for(halo_w):
                    sc = {'A': 0, 'T': 2, 'B': 4}[kind]
                    nrows = re - rs
                    if res is None and (rs, re) in dve_chunks:
                        zt_ = miscp.tile([C, 8 * L], BF16, tag="misc",
                                         name=f"az{ar_count[0]}_{rs}")
                        tt_ = miscp.tile([C, 8 * L], BF16, tag="misc",
                                         name=f"at{ar_count[0]}_{rs}")
                        zv = zt_[:].rearrange("p (r c) -> p r c", c=L)
                        tv = tt_[:].rearrange("p (r c) -> p r c", c=L)
                        nc.vector.tensor_scalar(
                            zv[:, 0:nrows, :], rawv[:, rs:re, :],
                            v[:, sc:sc + 1], v[:, sc + 1:sc + 2],
                            ALU.mult, ALU.add)
                        nc.vector.tensor_scalar(
                            tv[:, 0:nrows, :], zv[:, 0:nrows, :],
                            float(ALPHA), None, ALU.mult)
                        nc.vector.scalar_tensor_tensor(
                            xo[0:C, rs:re, H:H + L], zv[:, 0:nrows, :], 1.0,
                            tv[:, 0:nrows, :], ALU.mult, ALU.max)
                    elif res is None:
                        nc.scalar.activation(
                            xo[0:C, rs:re, H:H + L], rawv[:, rs:re, :],
                            AF.Lrelu, bias=v[:, sc + 1:sc + 2],
                            scale=v[:, sc:sc + 1], alpha=ALPHA)
                    else:
                        scr = miscp.tile([C, 8 * L], BF16, tag="misc",
                                         name=f"scr{ar_count[0]}_{rs}")
                        sv_ = scr[:].rearrange("p (r c) -> p r c", c=L)
                        nc.scalar.activation(
                            sv_[:, 0:nrows, :], rawv[:, rs:re, :],
                            AF.Lrelu, bias=v[:, sc + 1:sc + 2],
                            scale=v[:, sc:sc + 1], alpha=ALPHA)
                        rv = res[:].rearrange("p (r c) -> p r c", c=WC)
                        nc.vector.scalar_tensor_tensor(
                            xo[0:C, rs:re, H:H + L], sv_[:, 0:nrows, :], 1.0,
                            rv[0:C, rs:re, H:H + L], ALU.mult, ALU.add)
                    if d_next is not None:
                        crs, cre = max(rs, H), min(re, H + R)
                        if crs < cre:
                            nc.sync.dma_start(
                                xo[C:2 * C, crs - d_next:cre - d_next, :],
                                xo[0:C, crs:cre, :])

            def zero_pads(X):
                xo = X[:].rearrange("p (r c) -> p r c", c=WC)
                nc.vector.memset(xo[0:C, :, 0:H], 0.0)
                nc.vector.memset(xo[0:C, :, H + L:WC], 0.0)

            ROW_ORDER = [0, 1, 2, 3, 44, 45, 46, 47] + list(range(4, 44))

            # =========================================================
            # Phase A: x1 prenorm-fold, row/col, pair1 analytic stats
            # =========================================================
            rowf = wp.tile([C, R], F32, tag="rowf", name="rowf")
            colf = wp.tile([C, L], F32, tag="colf", name="colf")
            rowg = wp.tile([C, L], F32, tag="rowg", name="rowg")
            ps_col = pp.tile([C, L], F32, tag="ps", name="ps_col")
            ps_colb = pp.tile([C, L], F32, tag="ps", name="ps_colb")
            ps_row = pp.tile([C, L], F32, tag="ps", name="ps_row")
            for kk in range(7):
                st = x1wp.tile([128, L], F32, tag="x1st", name=f"x1st{kk}")
                nc.sync.dma_start(st[:], x1p[kk * 128:(kk + 1) * 128, :])
                s6 = small_vecs(8, 128)
                nc.vector.bn_stats(s6[:, 0:6], st[:])
                nc.vector.bn_aggr(s6[:, 6:8], s6[:, 0:6])
                sd = small_vecs(1, 128)
                rsqrt_cols(sd[:, 0:1], s6[:, 7:8])
                wsrc = x1wp.tile([128, 2 * C], F32, tag="w1src", name=f"w1src{kk}")
                nc.sync.dma_start(wsrc[:, 0:C], w1aT[kk * 128:(kk + 1) * 128, :])
                nc.sync.dma_start(wsrc[:, C:2 * C], w1bT[kk * 128:(kk + 1) * 128, :])
                wa = x1wp.tile([128, C], BF16, tag="w1a", name=f"w1a{kk}")
                wb = x1wp.tile([128, C], BF16, tag="w1b", name=f"w1b{kk}")
                nc.vector.tensor_scalar(wa[:], wsrc[:, 0:C], sd[:, 0:1], None, ALU.mult)
                nc.vector.tensor_scalar(wb[:], wsrc[:, C:2 * C], sd[:, 0:1], None, ALU.mult)
                xc = x1wp.tile([128, L], BF16, tag="x1c", name=f"x1c{kk}")
                nc.scalar.copy(xc[:], st[:])
                xlst = x1wp.tile([128, R], F32, tag="x1lst", name=f"x1lst{kk}")
                nc.sync.dma_start(xlst[:], x1loc[kk * 128:(kk + 1) * 128, :])
                xl = x1wp.tile([128, R], BF16, tag="x1l", name=f"x1l{kk}")
                nc.scalar.copy(xl[:], xlst[:])
                nc.tensor.matmul(ps_row[:, 0:R], wa[:], xl[:],
                                 start=(kk == 0), stop=(kk == 6))
                nc.tensor.matmul(ps_col[:, :], wa[:], xc[:],
                                 start=(kk == 0), stop=(kk == 6))
                nc.tensor.matmul(ps_colb[:, :], wb[:], xc[:],
                                 start=(kk == 0), stop=(kk == 6))
            nc.scalar.copy(rowf[:], ps_row[:, 0:R])
            nc.scalar.copy(colf[:], ps_colb[:, :])
            nc.scalar.copy(rowg[:], ps_col[:, :])

            # pair1 stats: var1 = var_r + var_c ; mu1 = m_r + m_c
            v1 = small_vecs()
            st6 = small_vecs(12)
            nc.vector.bn_stats(st6[:, 0:6], rowg[:])
            nc.vector.bn_aggr(st6[:, 6:8], st6[:, 0:6])
            nc.vector.bn_stats(st6[:, 0:6], colf[:])
            nc.vector.bn_aggr(st6[:, 8:10], st6[:, 0:6])
            nc.vector.scalar_tensor_tensor(
                st6[:, 10:11], st6[:, 6:7], 1.0, st6[:, 8:9], ALU.mult, ALU.add)
            nc.vector.scalar_tensor_tensor(
                st6[:, 11:12], st6[:, 7:8], 1.0, st6[:, 9:10], ALU.mult, ALU.add)
            rsqrt_cols(v1[:, 6:7], st6[:, 11:12])
            nc.vector.scalar_tensor_tensor(
                v1[:, 0:1], v1[:, 6:7], 1.0, gbt[:, 0:1], ALU.mult, ALU.mult)
            nc.vector.scalar_tensor_tensor(
                v1[:, 7:8], st6[:, 10:11], 1.0, v1[:, 0:1], ALU.mult, ALU.mult)
            nc.vector.scalar_tensor_tensor(
                v1[:, 1:2], v1[:, 7:8], -1.0, gbt[:, 1:2], ALU.mult, ALU.add)
            B1 = wp.tile([C, R], F32, tag="B1", name="B1")
            nc.vector.tensor_scalar(B1[:], rowf[:], v1[:, 0:1], v1[:, 1:2],
                                    ALU.mult, ALU.add)

            # =========================================================
            # Phase B: x2 stats pass + AllReduce + fold W2
            # =========================================================
            x2st_a = stgp.tile([128, 36 * 6], F32, tag="x2st", name="x2st_a")
            x2st_b = stgp.tile([128, 36 * 6], F32, tag="x2st", name="x2st_b")
            NS = 4 * L          # 4-row staging
            for ci, (x2p_, stt) in enumerate(((x2a, x2st_a), (x2b, x2st_b))):
                for g in range(12):
                    st = miscp.tile([128, NS], F32, tag="misc", name=f"x2s{ci}_{g}")
                    nc.sync.dma_start(st[:], x2p_[:, g * NS:(g + 1) * NS])
                    for sub in range(3):
                        nc.vector.bn_stats(
                            stt[:, (g * 3 + sub) * 6:(g * 3 + sub + 1) * 6],
                            st[:, sub * 512:(sub + 1) * 512])
            mv2 = small_vecs(8, 128)
            nc.vector.bn_aggr(mv2[:, 0:2], x2st_a[:].rearrange("p (a b) -> p a b", b=6))
            nc.vector.bn_aggr(mv2[:, 2:4], x2st_b[:].rearrange("p (a b) -> p a b", b=6))
            pl2 = small_vecs(4, 128)
            for (mc, vc, o) in ((0, 1, 0), (2, 3, 2)):
                nc.vector.tensor_scalar(pl2[:, o:o + 1], mv2[:, mc:mc + 1],
                                        0.125, None, ALU.mult)
                nc.vector.scalar_tensor_tensor(
                    mv2[:, 4 + o:5 + o], mv2[:, mc:mc + 1], 1.0,
                    mv2[:, mc:mc + 1], ALU.mult, ALU.mult)
                nc.vector.scalar_tensor_tensor(
                    mv2[:, 5 + o:6 + o], mv2[:, 4 + o:5 + o], 1.0,
                    mv2[:, vc:vc + 1], ALU.mult, ALU.add)
                nc.vector.tensor_scalar(pl2[:, o + 1:o + 2], mv2[:, 5 + o:6 + o],
                                        0.125, None, ALU.mult)
            nc.sync.dma_start(cc128i[:], pl2[:])
            nc.gpsimd.collective_compute(
                "AllReduce", ALU.add, replica_groups=groups,
                ins=[cc128i.opt()], outs=[cc128o.opt()])
            g2v = small_vecs(8, 128)
            nc.sync.dma_start(g2v[:, 0:4], cc128o[:])
            w2fa = wp.tile([128, C], F32, tag="w2fa", name="w2fa")
            w2fb = wp.tile([128, C], F32, tag="w2fb", name="w2fb")
            for (o, wt) in ((0, w2fa), (2, w2fb)):
                h = 4 + o
                nc.vector.scalar_tensor_tensor(
                    g2v[:, h:h + 1], g2v[:, o:o + 1], 1.0,
                    g2v[:, o:o + 1], ALU.mult, ALU.mult)
                nc.vector.scalar_tensor_tensor(
                    g2v[:, h:h + 1], g2v[:, h:h + 1],
                    -1.0, g2v[:, o + 1:o + 2], ALU.mult, ALU.add)
                rsqrt_cols(g2v[:, h + 1:h + 2], g2v[:, h:h + 1])
                wsrc2 = x1wp.tile([128, C], F32, tag="w2src", name=f"w2src{o}")
                nc.sync.dma_start(wsrc2[:], w2T[(o // 2) * 128:(o // 2 + 1) * 128, :])
                nc.vector.tensor_scalar(wt[:], wsrc2[:],
                                        g2v[:, h + 1:h + 2], None, ALU.mult)

            # =========================================================
            # Phase C: build z1 (48 rows)
            # =========================================================
            z1 = actp.tile([128, RT * WC], BF16, tag="act", name="z1")
            for r in range(R):
                nc.scalar.activation(
                    z1[0:C, r * L:(r + 1) * L], colf[:], AF.Lrelu,
                    bias=B1[:, r:r + 1], scale=v1[:, 0:1], alpha=ALPHA)

            # =========================================================
            # Phase D: pair2 matmul + stats + AR + z2
            # =========================================================
            stg2 = stgp.tile([C, R * 6], F32, tag="stg", name="stg_p2")
            for g in range(12):
                sta = miscp.tile([128, NS], F32, tag="misc", name=f"p2a{g}")
                stb = miscp.tile([128, NS], F32, tag="misc", name=f"p2b{g}")
                nc.sync.dma_start(sta[:], x2a[:, g * NS:(g + 1) * NS])
                nc.sync.dma_start(stb[:], x2b[:, g * NS:(g + 1) * NS])
                for rr in range(4):
                    r = g * 4 + rr
                    ps = pp.tile([C, L], F32, tag="ps", name=f"ps2_{r}")
                    nc.tensor.matmul(ps[:], w2fa[:],
                                     sta[:, rr * L:(rr + 1) * L],
                                     start=True, stop=False)
                    nc.tensor.matmul(ps[:], w2fb[:],
                                     stb[:, rr * L:(rr + 1) * L],
                                     start=False, stop=True)
                    nc.scalar.copy(rawv[:, r + H, :], ps[:])
                    nc.vector.bn_stats(stg2[:, r * 6:(r + 1) * 6], rawv[:, r + H, :])
            v2 = emit_stats_ar(stg2, R, gbt[:, 2:3], gbt[:, 3:4], masked=False)
            z2 = actp.tile([128, RT * WC], BF16, tag="act", name="z2")
            for g in range(6):
                nc.scalar.activation(
                    z2[0:C, g * 8 * L:(g + 1) * 8 * L],
                    rawv[:, g * 8 + H:(g + 1) * 8 + H, :],
                    AF.Lrelu, bias=v2[:, 1:2], scale=v2[:, 0:1], alpha=ALPHA)

            # =========================================================
            # Phase E: pair3 + first halo exchange + x0
            # =========================================================
            stg3 = stgp.tile([C, R * 6], F32, tag="stg", name="stg_p3")
            for i, r in enumerate(ROW_ORDER):
                ps = pp.tile([C, L], F32, tag="ps", name=f"ps3_{r}")
                nc.tensor.matmul(ps[:], w3t[:, 0:C], z1[0:C, r * L:(r + 1) * L],
                                 start=True, stop=False)
                nc.tensor.matmul(ps[:], w3t[:, C:2 * C], z2[0:C, r * L:(r + 1) * L],
                                 start=False, stop=True)
                nc.scalar.copy(rawv[:, r + H, :], ps[:])
                nc.vector.bn_stats(stg3[:, r * 6:(r + 1) * 6], rawv[:, r + H, :])
                if i == 7:
                    emit_halo_collective()
            v3 = emit_stats_ar(stg3, R, gbt[:, 4:5], gbt[:, 5:6], masked=True)
            emit_halo_gather()
            x0 = actp.tile([128, RT * WC], BF16, tag="act", name="x0")
            zero_pads(x0)
            emit_apply(v3, x0, d_next=CONV_D[0], halo_w=HW_X0)

            # =========================================================
            # Phase F: 10 conv stages
            # =========================================================
            X = x0
            res_t = x0
            for s in range(10):
                d = CONV_D[s]
                wofs = s * 3 * C
                xv = X[:].rearrange("p (r c) -> p r c", c=WC)
                stgs = stgp.tile([C, R * 6], F32, tag="stg", name=f"stg_s{s}")
                for i, r in enumerate(ROW_ORDER):
                    rho = r + H
                    ps = pp.tile([C, L], F32, tag="ps", name=f"ps_s{s}_{r}")
                    for ti, dc in ((0, -d), (1, 0), (2, d)):
                        nc.tensor.matmul(
                            ps[:], wpair_t[:, wofs + ti * C:wofs + (ti + 1) * C],
                            xv[:, rho - d, H + dc:H + dc + L],
                            start=(ti == 0), stop=False)
                    for ti, dc in ((0, -d), (1, 0), (2, d)):
                        nc.tensor.matmul(
                            ps[:], wsing_t[:, wofs + ti * C:wofs + (ti + 1) * C],
                            xv[0:C, rho + d, H + dc:H + dc + L],
                            start=False, stop=(ti == 2))
                    nc.scalar.copy(rawv[:, rho, :], ps[:])
                    nc.vector.bn_stats(stgs[:, r * 6:(r + 1) * 6], rawv[:, rho, :])
                    if i == 7 and s < 9:
                        emit_halo_collective()
                gamma = gbt[:, 6 + 2 * s:7 + 2 * s]
                beta = gbt[:, 7 + 2 * s:8 + 2 * s]
                vs = emit_stats_ar(stgs, R, gamma, beta, masked=(s < 9))
                if s < 9:
                    emit_halo_gather()
                if s == 9:
                    for (rs, re) in FINAL_CHUNKS:
                        scr = miscp.tile([C, NS], BF16, tag="misc",
                                         name=f"fscr{rs}")
                        so = scr[:].rearrange("p (r c) -> p r c", c=L)
                        nc.scalar.activation(
                            so[:, 0:re - rs, :], rawv[:, rs:re, :], AF.Lrelu,
                            bias=vs[:, 1:2], scale=vs[:, 0:1], alpha=ALPHA)
                        ost = miscp.tile([C, NS], F32, tag="misc",
                                         name=f"ost{rs}")
                        rv = res_t[:].rearrange("p (r c) -> p r c", c=WC)
                        nc.vector.scalar_tensor_tensor(
                            ost[:].rearrange("p (r c) -> p r c", c=L),
                            so[:, 0:re - rs, :], 1.0,
                            rv[0:C, rs:re, H:H + L], ALU.mult, ALU.add)
                        nc.sync.dma_start(
                            out_p[:, (rs - H) * L:(re - H) * L], ost[:])
                else:
                    Xn = actp.tile([128, RT * WC], BF16, tag="act", name=f"x_s{s}")
                    zero_pads(Xn)
                    if s % 2 == 0:
                        emit_apply(vs, Xn, d_next=NEXT_D[s], halo_w=HW_OUT[s])
                        res_new = res_t
                    else:
                        emit_apply(vs, Xn, d_next=NEXT_D[s], res=res_t,
                                   halo_w=HW_OUT[s])
                        res_new = Xn
                    X = Xn
                    res_t = res_new
    return nc


_CACHED = {}


def _host_prep(x_1d, x_2d, W1, g1, b1, W2, g2, b2, W3, g3, b3,
               res_w, res_b, res_g, res_beta):
    bf = ml_dtypes.bfloat16
    x1 = np.asarray(x_1d, np.float32)[0]
    x2 = np.asarray(x_2d, np.float32)[0]
    x1p = np.zeros((D1P, L), np.float32); x1p[:D1] = x1
    w1aT = np.zeros((D1P, C), np.float32); w1aT[:D1] = np.asarray(W1, np.float32)[:, :D1].T
    w1bT = np.zeros((D1P, C), np.float32); w1bT[:D1] = np.asarray(W1, np.float32)[:, D1:].T
    w2T = np.zeros((D2P, C), np.float32); w2T[:D2] = np.asarray(W2, np.float32).T
    W3_ = np.asarray(W3, np.float32)
    w3t = np.ascontiguousarray(
        np.concatenate([W3_[:, :C].T, W3_[:, C:].T], axis=1)).astype(bf)
    rw = np.asarray(res_w, np.float32)
    wpair = np.zeros((128, 10 * 3 * C), np.float32)
    wsing = np.zeros((C, 10 * 3 * C), np.float32)
    for s in range(10):
        W = rw[s // 2, s % 2]                                # [O, I, 3, 3]
        for ti in range(3):
            o = (s * 3 + ti) * C
            wpair[0:C, o:o + C] = W[:, :, 0, ti].T
            wpair[C:128, o:o + C] = W[:, :, 1, ti].T
            wsing[:, o:o + C] = W[:, :, 2, ti].T
    gb = np.zeros((C, 26), np.float32)
    pg = [(g1, b1), (g2, b2), (g3, b3)] + [
        (np.asarray(res_g)[s // 2, s % 2], np.asarray(res_beta)[s // 2, s % 2])
        for s in range(10)]
    for i, (g, b) in enumerate(pg):
        gb[:, 2 * i] = np.asarray(g, np.float32)
        gb[:, 2 * i + 1] = np.asarray(b, np.float32)

    in_maps = []
    for i in range(NCORES):
        r0 = i * R
        x2sh = x2[:, r0:r0 + R, :]
        x2a_ = np.ascontiguousarray(x2sh[0:128].reshape(128, R * L))
        x2b_ = np.zeros((128, R * L), np.float32)
        x2b_[0:D2 - 128] = x2sh[128:D2].reshape(D2 - 128, R * L)
        up = i - 1 if i > 0 else 0
        dn = i + 1 if i < 7 else 0
        p = np.arange(C)
        hidx = np.stack([up * 2 * C + p * 2 + 1, dn * 2 * C + p * 2 + 0],
                        axis=1).astype(np.int32)
        masks = np.ones((C, 2), np.float32)
        if i == 0:
            masks[:, 0] = 0.0
        if i == NCORES - 1:
            masks[:, 1] = 0.0
        in_maps.append(dict(
            x1p=x1p, x1loc=np.ascontiguousarray(x1p[:, r0:r0 + R]),
            x2a=x2a_, x2b=x2b_, w1aT=w1aT, w1bT=w1bT, w2T=w2T,
            w3t=w3t, wpair=wpair.astype(bf), wsing=wsing.astype(bf),
            gb=gb, masks=masks, hidx=hidx))
    return in_maps


def kernel(**inputs):
    in_maps = _host_prep(**inputs)
    if "nc" not in _CACHED:
        _CACHED["nc"] = build()
        split_multi_waits(_CACHED["nc"])
    res = run_bass_kernel_spmd(_CACHED["nc"], in_maps, list(range(NCORES)),
                               trace=False)
    out = np.concatenate(
        [res.results[i]["out"].reshape(C, R, L) for i in range(NCORES)], axis=1)
    return out[None].astype(np.float32)
